# revision 1
# baseline (speedup 1.0000x reference)
"""GCN (2-layer GCNConv + linear head) distributed over 8 TRN2 NeuronCores.

Strategy (graph/data parallel, dst-partitioned):
  - Nodes are partitioned into 8 contiguous ranges (one per core); each core
    owns the output rows (scatter destinations) for its range.
  - Per-edge messages are gathered from a full node-feature table in DRAM via
    `dma_gather` (512B rows), scattered into per-dst-tile accumulators with a
    one-hot matmul on the TensorEngine:
        aggT[f, d] += X_chunk.T @ onehot_chunk        (PSUM accumulate)
    where onehot[e, d] = (d == dst_e) * norm_e is built on the VectorEngine in
    one tensor_scalar op (is_equal then mult against per-partition scalars).
  - GCN normalization (dinv[s]*dinv[d]) is folded into the one-hot payload;
    self-loops are handled as one diagonal "cell" per tile reading the core's
    local slice contiguously (no gather needed).
  - Layer math in transposed space: h_T = relu(W.T @ aggT + b); only layer-1
    output is transposed back (PE transpose) and written node-major so the
    inter-layer AllGather output can serve as layer-2's gather table.
  - One AllGather (8 cores, ~51MB f32) between the layers.
  - dma_gather indices are int16, so the table is read through 4 windows of
    NSLOT/4 rows; edges are bucketed by (dst tile, src window) on the host.

All host-side prep (degree/norm computation, edge bucketing, padding) is in
numpy inside kernel(); the device kernel is a single static SPMD program, so
per-(tile,window) chunk counts are maxed across cores.
"""

import math
import os
import sys

import numpy as np

for _p in ("/opt/trn_rl_repo",):
    if _p not in sys.path and os.path.isdir(_p):
        sys.path.insert(0, _p)

# ---------------------------------------------------------------- config ----

F = 128  # feature/hidden width


class Cfg:
    def __init__(self, n_cores=8, nodes_real_per_core=12500, n_edges=1_600_000,
                 n_windows=4, gather_block=1024, single_packet=True):
        self.SP = single_packet
        self.C = n_cores
        self.NR = nodes_real_per_core
        self.T = (self.NR + 127) // 128          # dst tiles per core
        self.S = self.T * 128                    # node slots per core
        self.NSLOT = self.C * self.S             # global slot count
        self.NW = n_windows
        assert self.NSLOT % self.NW == 0
        self.WIN = self.NSLOT // self.NW         # rows per gather window
        assert self.WIN <= 32767, "dma_gather idx is int16"
        self.GB = gather_block                   # idxs per dma_gather
        assert self.GB % 128 == 0
        self.N = self.C * self.NR                # real node count
        self.E = n_edges


FULL = Cfg(gather_block=4096, single_packet=False)


# ------------------------------------------------------------- host prep ----

def prepare(cfg: Cfg, x, edge_index):
    """Compute per-core device inputs (except weights) + shared static layout.

    Returns (layout, per_core_arrays) where layout has the shared chunk
    schedule and per_core_arrays is a list of dicts of numpy arrays.
    """
    C, NR, T, S, NW, WIN, GB = cfg.C, cfg.NR, cfg.T, cfg.S, cfg.NW, cfg.WIN, cfg.GB
    N = cfg.N
    src = np.asarray(edge_index[0], dtype=np.int64)
    dst = np.asarray(edge_index[1], dtype=np.int64)
    x = np.asarray(x, dtype=np.float32)

    deg = np.bincount(dst, minlength=N).astype(np.float64) + 1.0  # + self loop
    dinv = (1.0 / np.sqrt(deg)).astype(np.float32)

    norm = dinv[src] * dinv[dst]

    core_of = dst // NR
    s_slot = S * (src // NR) + (src % NR)
    d_slot = S * core_of + (dst % NR)
    t_loc = (d_slot % S) // 128
    d_loc = (d_slot % 128).astype(np.float32)
    w_of = s_slot // WIN

    # sort edges by (core, tile, window, src-slot)
    order = np.lexsort((s_slot, w_of, t_loc, core_of))
    s_slot, d_loc, norm = s_slot[order], d_loc[order], norm[order]
    core_s, t_s, w_s = core_of[order], t_loc[order], w_of[order]

    cell = ((core_s * T + t_s) * NW + w_s).astype(np.int64)
    counts = np.bincount(cell, minlength=C * T * NW).reshape(C, T, NW)
    Kcell = (np.ceil(counts / 128.0).astype(np.int64)).max(axis=0)  # [T, NW]
    cell_starts = np.zeros(C * T * NW + 1, dtype=np.int64)
    np.cumsum(np.bincount(cell, minlength=C * T * NW), out=cell_starts[1:])

    # chunk schedule (shared across cores): window-major, then tile
    # chunk_of_cell[w][t] = first global chunk index of cell (t, w)
    Ctot = int(Kcell.sum())
    chunk_base = np.zeros((NW, T), dtype=np.int64)
    acc = 0
    for w in range(NW):
        for t in range(T):
            chunk_base[w, t] = acc
            acc += int(Kcell[t, w])
    assert acc == Ctot
    Lw = [int(Kcell[:, w].sum()) * 128 for w in range(NW)]  # idx per window

    per_core = []
    for c in range(C):
        idx_streams = [np.zeros(Lw[w], dtype=np.int16) for w in range(NW)]
        dst_stream = np.full(Ctot * 128, -1.0, dtype=np.float32)
        norm_stream = np.zeros(Ctot * 128, dtype=np.float32)
        for w in range(NW):
            wchunk0 = chunk_base[w, 0] - (chunk_base[0, 0] if False else chunk_base[w, 0])
            for t in range(T):
                ci = (c * T + t) * NW + w
                e0, e1 = cell_starts[ci], cell_starts[ci + 1]
                n = e1 - e0
                # position inside this window's idx stream
                woff = int((chunk_base[w, t] - chunk_base[w, 0]) * 128)
                idx_streams[w][woff:woff + n] = (s_slot[e0:e1] - w * WIN).astype(np.int16)
                # global chunk stream position for dst/norm
                goff = int(chunk_base[w, t]) * 128
                dst_stream[goff:goff + n] = d_loc[e0:e1]
                norm_stream[goff:goff + n] = norm[e0:e1]

        # wrap idx into [128, L/16] (edge i -> [i%16, i//16], replicated x8)
        idx_wrapped = []
        for w in range(NW):
            a = idx_streams[w].reshape(-1, 16).T  # [16, L/16]
            idx_wrapped.append(np.tile(a, (8, 1)).copy())  # [128, L/16]

        dst_t = dst_stream.reshape(Ctot, 128).T.copy()    # [128, Ctot]
        norm_t = norm_stream.reshape(Ctot, 128).T.copy()  # [128, Ctot]

        # dinv^2 per local slot (0 for pad slots)
        d2 = np.zeros(S, dtype=np.float32)
        d2[:NR] = dinv[c * NR:(c + 1) * NR] ** 2
        dinv2_t = d2.reshape(T, 128).T.copy()             # [128, T]

        per_core.append(dict(
            idx_wrapped=idx_wrapped, dst_t=dst_t, norm_t=norm_t, dinv2_t=dinv2_t,
        ))

    # x in slot space
    x_slot = np.zeros((cfg.NSLOT, F), dtype=np.float32)
    sl = S * (np.arange(N) // NR) + (np.arange(N) % NR)
    x_slot[sl] = x
    for c in range(C):
        per_core[c]["x_tab"] = x_slot
        per_core[c]["x_loc"] = x_slot[c * S:(c + 1) * S].copy()

    layout = dict(Kcell=Kcell, chunk_base=chunk_base, Lw=Lw, Ctot=Ctot)
    return layout, per_core


# ---------------------------------------------------------------- builder ----

def build_nc(cfg: Cfg, layout):
    import concourse.bacc as bacc
    import concourse.mybir as mybir
    import concourse.tile as tile

    dtf = mybir.dt.float32
    Relu = mybir.ActivationFunctionType.Relu
    EQ = mybir.AluOpType.is_equal
    MUL = mybir.AluOpType.mult
    ADD = mybir.AluOpType.add

    C, T, S, NW, WIN, GB = cfg.C, cfg.T, cfg.S, cfg.NW, cfg.WIN, cfg.GB
    Kcell, chunk_base, Lw, Ctot = (layout["Kcell"], layout["chunk_base"],
                                   layout["Lw"], layout["Ctot"])

    nc = bacc.Bacc("TRN2", target_bir_lowering=False, debug=False,
                   num_devices=C)

    x_tab = nc.dram_tensor("x_tab", [cfg.NSLOT, F], dtf, kind="ExternalInput").ap()
    x_loc = nc.dram_tensor("x_loc", [S, F], dtf, kind="ExternalInput").ap()
    idx_d = [nc.dram_tensor(f"idx_w{w}", [128, Lw[w] // 16], mybir.dt.int16,
                            kind="ExternalInput").ap() for w in range(NW)]
    dst_d = nc.dram_tensor("dst_t", [128, Ctot], dtf, kind="ExternalInput").ap()
    norm_d = nc.dram_tensor("norm_t", [128, Ctot], dtf, kind="ExternalInput").ap()
    dinv2_d = nc.dram_tensor("dinv2_t", [128, T], dtf, kind="ExternalInput").ap()
    iota_d = nc.dram_tensor("iota_row", [128, F], dtf, kind="ExternalInput").ap()
    iotac_d = nc.dram_tensor("iota_col", [128, 1], dtf, kind="ExternalInput").ap()
    ident_d = nc.dram_tensor("ident", [128, 128], dtf, kind="ExternalInput").ap()
    W1_d = nc.dram_tensor("W1", [F, F], dtf, kind="ExternalInput").ap()
    W2_d = nc.dram_tensor("W2", [F, F], dtf, kind="ExternalInput").ap()
    Wl_d = nc.dram_tensor("Wl", [F, 1], dtf, kind="ExternalInput").ap()
    b1_d = nc.dram_tensor("b1", [F, 1], dtf, kind="ExternalInput").ap()
    b2_d = nc.dram_tensor("b2", [F, 1], dtf, kind="ExternalInput").ap()
    bl_d = nc.dram_tensor("bl", [1, 1], dtf, kind="ExternalInput").ap()
    out_d = nc.dram_tensor("out", [1, S], dtf, kind="ExternalOutput").ap()

    with tile.TileContext(nc) as tc:
        with (
            tc.tile_pool(name="const", bufs=1) as const,
            tc.tile_pool(name="sb", bufs=2) as sb,
            tc.tile_pool(name="ohp", bufs=4) as ohp,
            tc.tile_pool(name="psum", bufs=1, space="PSUM") as psum,
            tc.tile_pool(name="pcell", bufs=3, space="PSUM") as pcell,
            tc.tile_pool(name="dram", bufs=1, space="DRAM") as dram,
        ):
            # constants
            iota_row = const.tile([128, F], dtf)
            nc.sync.dma_start(iota_row[:], iota_d)
            iota_col = const.tile([128, 1], dtf)
            nc.sync.dma_start(iota_col[:], iotac_d)
            ident = const.tile([128, 128], dtf)
            nc.sync.dma_start(ident[:], ident_d)
            W1s = const.tile([F, F], dtf)
            nc.sync.dma_start(W1s[:], W1_d)
            W2s = const.tile([F, F], dtf)
            nc.sync.dma_start(W2s[:], W2_d)
            Wls = const.tile([F, 1], dtf)
            nc.sync.dma_start(Wls[:], Wl_d)
            b1s = const.tile([F, 1], dtf)
            nc.sync.dma_start(b1s[:], b1_d)
            b2s = const.tile([F, 1], dtf)
            nc.sync.dma_start(b2s[:], b2_d)
            bls = const.tile([1, 1], dtf)
            nc.sync.dma_start(bls[:], bl_d)
            dinv2s = const.tile([128, T], dtf)
            nc.sync.dma_start(dinv2s[:], dinv2_d)
            dsts = const.tile([128, Ctot], dtf)
            nc.sync.dma_start(dsts[:], dst_d)
            norms = const.tile([128, Ctot], dtf)
            nc.sync.dma_start(norms[:], norm_d)

            aggT = const.tile([128, T * F], dtf)   # [f, dst-slot] accumulators
            outsb = const.tile([1, S], dtf)

            h1_loc = dram.tile([S, F], dtf)
            ag_tab = dram.tile([cfg.NSLOT, F], dtf, addr_space="Shared")

            for layer in range(2):
                table = x_tab if layer == 0 else ag_tab[:]
                local = x_loc if layer == 0 else h1_loc[:]
                Ws = W1s if layer == 0 else W2s
                bs = b1s if layer == 0 else b2s

                # self-loop cells: aggT[:, t] = x_local_tile.T @ diag(dinv^2)
                for t in range(T):
                    xl = sb.tile([128, F], dtf, tag="xl")
                    nc.sync.dma_start(xl[:], local[t * 128:(t + 1) * 128, :])
                    soh = ohp.tile([128, F], dtf, tag="soh")
                    nc.vector.tensor_tensor(
                        out=soh[:], in0=iota_row[:],
                        in1=iota_col[:].to_broadcast([128, F]), op=EQ)
                    nc.vector.tensor_tensor(
                        out=soh[:], in0=soh[:],
                        in1=dinv2s[:, t:t + 1].to_broadcast([128, F]), op=MUL)
                    ps = pcell.tile([128, F], dtf, tag="ps_cell", name="ps")
                    nc.tensor.matmul(out=ps[:], lhsT=xl[:], rhs=soh[:],
                                     start=True, stop=True)
                    nc.scalar.copy(out=aggT[:, t * F:(t + 1) * F], in_=ps[:])

                # gathered edge cells, window-major
                for w in range(NW):
                    nwchunks = Lw[w] // 128
                    tbl = table[w * WIN:(w + 1) * WIN, :]
                    xb = None
                    for t in range(T):
                        K = int(Kcell[t, w])
                        if K == 0:
                            continue
                        pst = pcell.tile([128, F], dtf, tag="ps_cell")
                        for k in range(K):
                            jw = int(chunk_base[w, t] - chunk_base[w, 0]) + k
                            b, slot = divmod(jw, GB // 128)
                            if slot == 0:
                                blk = min(GB, (nwchunks - b * (GB // 128)) * 128)
                                it = sb.tile([128, GB // 16], mybir.dt.int16,
                                             tag="it")
                                nc.sync.dma_start(
                                    it[:, :blk // 16],
                                    idx_d[w][:, b * (GB // 16):
                                             b * (GB // 16) + blk // 16])
                                xb = sb.tile([128, GB // 128, F], dtf, tag="xb")
                                nc.gpsimd.dma_gather(
                                    xb[:, :blk // 128, :], tbl,
                                    it[:, :blk // 16], blk, blk, F,
                                    single_packet=cfg.SP)
                            gch = int(chunk_base[w, t]) + k  # global chunk id
                            oh = ohp.tile([128, F], dtf, tag="oh")
                            nc.vector.tensor_tensor(
                                out=oh[:], in0=iota_row[:],
                                in1=dsts[:, gch:gch + 1].to_broadcast([128, F]),
                                op=EQ)
                            nc.vector.tensor_tensor(
                                out=oh[:], in0=oh[:],
                                in1=norms[:, gch:gch + 1].to_broadcast([128, F]),
                                op=MUL)
                            nc.tensor.matmul(out=pst[:], lhsT=xb[:, slot, :],
                                             rhs=oh[:], start=(k == 0),
                                             stop=(k == K - 1))
                        nc.vector.tensor_add(out=aggT[:, t * F:(t + 1) * F],
                                             in0=aggT[:, t * F:(t + 1) * F],
                                             in1=pst[:])

                # per-tile transform
                for t in range(T):
                    p2 = psum.tile([128, F], dtf, tag="p2", bufs=2)
                    nc.tensor.matmul(out=p2[:], lhsT=Ws[:],
                                     rhs=aggT[:, t * F:(t + 1) * F],
                                     start=True, stop=True)
                    if layer == 0:
                        h1t = sb.tile([128, F], dtf, tag="h1t")
                        nc.scalar.activation(out=h1t[:], in_=p2[:], func=Relu,
                                             bias=b1s[:])
                        p3 = psum.tile([128, F], dtf, tag="p3")
                        nc.tensor.transpose(out=p3[:], in_=h1t[:],
                                            identity=ident[:])
                        h1 = sb.tile([128, F], dtf, tag="h1")
                        nc.vector.tensor_copy(out=h1[:], in_=p3[:])
                        nc.sync.dma_start(h1_loc[t * 128:(t + 1) * 128, :],
                                          h1[:])
                    else:
                        h2t = sb.tile([128, F], dtf, tag="h2t")
                        nc.scalar.activation(out=h2t[:], in_=p2[:], func=Relu,
                                             bias=b2s[:])
                        p4 = psum.tile([1, F], dtf, tag="p4")
                        nc.tensor.matmul(out=p4[:], lhsT=Wls[:], rhs=h2t[:],
                                         start=True, stop=True)
                        nc.vector.tensor_scalar(
                            out=outsb[:, t * 128:(t + 1) * 128], in0=p4[:],
                            scalar1=bls[:], scalar2=None, op0=ADD)

                if layer == 0:
                    nc.gpsimd.collective_compute(
                        "AllGather", mybir.AluOpType.bypass,
                        replica_groups=[list(range(C))],
                        ins=[h1_loc[:]], outs=[ag_tab[:]])

            nc.sync.dma_start(out_d, outsb[:])

    nc.compile()
    return nc


# ------------------------------------------------------------------ entry ----

def make_in_maps(cfg, per_core, W1, b1, W2, b2, Wl, bl):
    maps = []
    for c in range(cfg.C):
        pc = per_core[c]
        m = dict(
            x_tab=pc["x_tab"], x_loc=pc["x_loc"],
            dst_t=pc["dst_t"], norm_t=pc["norm_t"], dinv2_t=pc["dinv2_t"],
            W1=np.asarray(W1, np.float32), W2=np.asarray(W2, np.float32),
            Wl=np.asarray(Wl, np.float32).reshape(F, 1),
            b1=np.asarray(b1, np.float32).reshape(F, 1),
            b2=np.asarray(b2, np.float32).reshape(F, 1),
            bl=np.asarray(bl, np.float32).reshape(1, 1),
            iota_row=np.tile(np.arange(F, dtype=np.float32), (128, 1)),
            iota_col=np.arange(128, dtype=np.float32).reshape(128, 1),
            ident=np.eye(128, dtype=np.float32),
        )
        for w in range(cfg.NW):
            m[f"idx_w{w}"] = pc["idx_wrapped"][w]
        maps.append(m)
    return maps


def run(cfg, x, edge_index, W1, b1, W2, b2, Wl, bl, trace=False, nc=None):
    from concourse import bass_utils

    layout, per_core = prepare(cfg, x, edge_index)
    if nc is None:
        nc = build_nc(cfg, layout)
    in_maps = make_in_maps(cfg, per_core, W1, b1, W2, b2, Wl, bl)
    res = bass_utils.run_bass_kernel_spmd(nc, in_maps,
                                          core_ids=list(range(cfg.C)),
                                          trace=trace)
    out = np.concatenate([res.results[c]["out"][0, :cfg.NR]
                          for c in range(cfg.C)])
    return out.astype(np.float32), res


def kernel(x, edge_index, W1, b1, W2, b2, Wl, bl):
    out, _ = run(FULL, x, edge_index, W1, b1, W2, b2, Wl, bl)
    return out



# revision 4
# speedup vs baseline: 1.1907x; 1.1907x over previous
"""GCN (2-layer GCNConv + linear head) distributed over 8 TRN2 NeuronCores.

v2 strategy (dst-partitioned graph parallel, fp16 hot path, 4 SWDGE queues):
  - Nodes partitioned into 8 contiguous ranges; core c owns the scatter
    destinations for its range. Per-edge messages gathered from a node-major
    fp16 table in DRAM via dma_gather (256B rows) round-robined over 4 SWDGE
    queues (4x the single-queue drain rate).
  - Scatter via one-hot matmul on the TensorEngine: for each chunk of 128
    gathered edges, rhs[e, d] = (d == dst_e) * norm_e is built in ONE fused
    DVE op (tensor_scalar: is_equal then mult against per-partition scalars),
    and psum[f, d] += xb_chunk.T @ rhs accumulates in PSUM f32.
  - Cells are (supertile=256 dsts, window=src range); chunks padded to 128
    per cell with counts maxed across cores (shared SPMD schedule). The
    idx/dst/norm streams are shared by both layers (same edge pattern) and
    kept resident in SBUF.
  - Self-loops skip the gather: per block of <=5 supertiles the local x
    slice is bulk-loaded and scattered with a diagonal one-hot (dinv^2).
  - Per-block pipeline: up to 5 supertile PSUM accumulators (1 bank each)
    live across the block's windows; transforms chase the accumulation.
    Layer-1 output is produced NODE-major directly (lhsT=agg, rhs=W; bias
    via a rank-1 accumulate matmul) so no transpose is needed before the
    inter-layer AllGather; layer 2 stays feature-major for the linear head.
  - One AllGather (fp16, ~25MB) between the layers.

Host prep (degree/norm computation, edge bucketing, padding) is numpy inside
kernel(); the device kernel is a single static SPMD program.
"""

import math
import os
import sys

import numpy as np

for _p in ("/opt/trn_rl_repo",):
    if _p not in sys.path and os.path.isdir(_p):
        sys.path.insert(0, _p)

F = 128  # feature/hidden width
STW = 256  # dsts per supertile (2 tiles)


class Cfg:
    def __init__(self, n_cores=8, nodes_real_per_core=12500, n_edges=1_600_000,
                 n_windows=4, bst_max=5, gather_rows=8192):
        self.C = n_cores
        self.NR = nodes_real_per_core
        self.T = (self.NR + 127) // 128          # dst tiles per core
        assert self.T % 2 == 0, "supertile=2 tiles needs even T"
        self.NST = self.T // 2                   # supertiles per core
        self.S = self.T * 128                    # node slots per core
        self.NSLOT = self.C * self.S             # global slot count
        self.NW = n_windows
        assert self.NSLOT % self.NW == 0
        self.WIN = self.NSLOT // self.NW         # rows per gather window
        assert self.WIN <= 32767, "dma_gather idx is int16"
        self.BSTM = bst_max                      # max supertiles per block
        self.BLOCKS = []
        r = self.NST
        while r > 0:
            self.BLOCKS.append(min(bst_max, r))
            r -= min(bst_max, r)
        self.GB = gather_rows                    # max rows per dma_gather
        assert self.GB % 128 == 0
        self.N = self.C * self.NR                # real node count
        self.E = n_edges


FULL = Cfg()


# ------------------------------------------------------------- host prep ----

def prepare(cfg: Cfg, x, edge_index):
    """Compute per-core device inputs + the shared static schedule."""
    C, NR, T, S, NW, WIN = cfg.C, cfg.NR, cfg.T, cfg.S, cfg.NW, cfg.WIN
    NST, GB = cfg.NST, cfg.GB
    N = cfg.N
    src = np.asarray(edge_index[0], dtype=np.int64)
    dst = np.asarray(edge_index[1], dtype=np.int64)
    x = np.asarray(x, dtype=np.float32)

    deg = np.bincount(dst, minlength=N).astype(np.float64) + 1.0  # + self loop
    dinv = (1.0 / np.sqrt(deg)).astype(np.float32)

    norm = (dinv[src] * dinv[dst]).astype(np.float32)

    core_of = dst // NR
    s_slot = S * (src // NR) + (src % NR)
    d_in_core = dst % NR
    st_of = d_in_core // STW
    d_loc = (d_in_core - st_of * STW).astype(np.float32)  # 0..255
    w_of = s_slot // WIN

    order = np.lexsort((s_slot, w_of, st_of, core_of))
    s_slot, d_loc, norm = s_slot[order], d_loc[order], norm[order]
    core_s, st_s, w_s = core_of[order], st_of[order], w_of[order]

    cell = ((core_s * NST + st_s) * NW + w_s).astype(np.int64)
    counts = np.bincount(cell, minlength=C * NST * NW).reshape(C, NST, NW)
    Kcell = np.ceil(counts / 128.0).astype(np.int64).max(axis=0)  # [NST, NW]
    cell_starts = np.zeros(C * NST * NW + 1, dtype=np.int64)
    np.cumsum(np.bincount(cell, minlength=C * NST * NW), out=cell_starts[1:])

    # ---- shared schedule: blocks x windows -> calls of <= GB/128 chunks ----
    # chunk_pos[st, w] = global chunk index of cell (st, w)'s first chunk,
    # in processing order (block-major, then window, then supertile).
    chunk_pos = np.zeros((NST, NW), dtype=np.int64)
    calls = []  # (block_idx, w, col0_chunk, [(st, k)...]) per gather call
    acc = 0
    st0 = 0
    for bi, bsz in enumerate(cfg.BLOCKS):
        sts = range(st0, st0 + bsz)
        for w in range(NW):
            pend = []
            for st in sts:
                chunk_pos[st, w] = acc + len(pend)
                pend += [(st, k) for k in range(int(Kcell[st, w]))]
            i = 0
            while i < len(pend):
                n = min(GB // 128, len(pend) - i)
                calls.append((bi, w, acc + i, pend[i:i + n]))
                i += n
            acc += len(pend)
        st0 += bsz
    Ctot = acc
    assert Ctot == int(Kcell.sum())

    # last (w, k) per supertile for the psum stop flag
    last_of_st = {}
    for st in range(NST):
        ws = [w for w in range(NW) if Kcell[st, w] > 0]
        if ws:
            w = ws[-1]
            last_of_st[st] = (w, int(Kcell[st, w]) - 1)

    # ---- per-core streams ----
    per_core = []
    for c in range(C):
        idxs = np.zeros(Ctot * 128, dtype=np.int16)
        dstv = np.full(Ctot * 128, -1.0, dtype=np.float32)
        nrmv = np.zeros(Ctot * 128, dtype=np.float32)
        for st in range(NST):
            for w in range(NW):
                K = int(Kcell[st, w])
                if K == 0:
                    continue
                ci = (c * NST + st) * NW + w
                e0, e1 = cell_starts[ci], cell_starts[ci + 1]
                n = int(e1 - e0)
                off = int(chunk_pos[st, w]) * 128
                idxs[off:off + n] = (s_slot[e0:e1] - w_s[e0:e1] * WIN).astype(np.int16)
                dstv[off:off + n] = d_loc[e0:e1]
                nrmv[off:off + n] = norm[e0:e1]

        # idx wrapped [128, Ctot*8]: idx i -> [i%16, i//16], replicated x8
        idx_w = np.tile(idxs.reshape(-1, 16).T, (8, 1)).copy()
        dst_t = dstv.reshape(Ctot, 128).T.copy()
        nrm_t = nrmv.reshape(Ctot, 128).T.copy()

        d2 = np.zeros(S, dtype=np.float32)
        d2[:NR] = dinv[c * NR:(c + 1) * NR] ** 2
        dinv2_t = d2.reshape(T, 128).T.copy()  # [128, T] f32

        per_core.append(dict(idx_w=idx_w, dst_t=dst_t, nrm_t=nrm_t,
                             dinv2_t=dinv2_t))

    # node tables in slot space (fp16)
    x_slot = np.zeros((cfg.NSLOT, F), dtype=np.float16)
    sl = S * (np.arange(N) // NR) + (np.arange(N) % NR)
    x_slot[sl] = x.astype(np.float16)
    for c in range(C):
        per_core[c]["xtab"] = x_slot
        per_core[c]["xloc"] = x_slot[c * S:(c + 1) * S].copy()

    layout = dict(Kcell=Kcell, calls=calls, chunk_pos=chunk_pos,
                  last_of_st=last_of_st, Ctot=Ctot)
    return layout, per_core


# ---------------------------------------------------------------- builder ----

def build_nc(cfg: Cfg, layout):
    import concourse.bacc as bacc
    import concourse.mybir as mybir
    import concourse.tile as tile

    dtf = mybir.dt.float32
    dth = mybir.dt.float16
    Relu = mybir.ActivationFunctionType.Relu
    EQ = mybir.AluOpType.is_equal
    MUL = mybir.AluOpType.mult
    ADD = mybir.AluOpType.add

    C, T, S, NW, WIN, GB = cfg.C, cfg.T, cfg.S, cfg.NW, cfg.WIN, cfg.GB
    NST = cfg.NST
    Kcell, calls, chunk_pos, last_of_st, Ctot = (
        layout["Kcell"], layout["calls"], layout["chunk_pos"],
        layout["last_of_st"], layout["Ctot"])

    nc = bacc.Bacc("TRN2", target_bir_lowering=False, debug=False,
                   num_devices=C, num_swdge_queues=4)

    xtab_d = nc.dram_tensor("xtab", [cfg.NSLOT, F], dth, kind="ExternalInput").ap()
    xloc_d = nc.dram_tensor("xloc", [S, F], dth, kind="ExternalInput").ap()
    idx_d = nc.dram_tensor("idx_w", [128, Ctot * 8], mybir.dt.int16,
                           kind="ExternalInput").ap()
    dst_d = nc.dram_tensor("dst_t", [128, Ctot], dtf, kind="ExternalInput").ap()
    nrm_d = nc.dram_tensor("nrm_t", [128, Ctot], dtf, kind="ExternalInput").ap()
    dinv2_d = nc.dram_tensor("dinv2_t", [128, T], dtf, kind="ExternalInput").ap()
    iota_d = nc.dram_tensor("iota256", [128, STW], dth, kind="ExternalInput").ap()
    iotac_d = nc.dram_tensor("iota_col2", [128, 2], dtf, kind="ExternalInput").ap()
    W1_d = nc.dram_tensor("W1", [F, F], dth, kind="ExternalInput").ap()
    W2_d = nc.dram_tensor("W2", [F, F], dth, kind="ExternalInput").ap()
    Wl_d = nc.dram_tensor("Wl", [F, 1], dth, kind="ExternalInput").ap()
    ones_d = nc.dram_tensor("ones1", [1, 128], dth, kind="ExternalInput").ap()
    b1r_d = nc.dram_tensor("b1row", [1, F], dth, kind="ExternalInput").ap()
    b2_d = nc.dram_tensor("b2", [F, 1], dtf, kind="ExternalInput").ap()
    bl_d = nc.dram_tensor("bl", [1, 1], dtf, kind="ExternalInput").ap()
    out_d = nc.dram_tensor("out", [1, S], dtf, kind="ExternalOutput").ap()

    with tile.TileContext(nc) as tc:
        with (
            tc.tile_pool(name="const", bufs=1) as const,
            tc.tile_pool(name="sb", bufs=2) as sb,
            tc.tile_pool(name="ohp", bufs=6) as ohp,
            tc.tile_pool(name="aggp", bufs=1, space="PSUM") as aggp,
            tc.tile_pool(name="pt", bufs=1, space="PSUM") as pt,
            tc.tile_pool(name="dram", bufs=1, space="DRAM") as dram,
        ):
            iota256 = const.tile([128, STW], dth)
            nc.sync.dma_start(iota256[:], iota_d)
            iotac = const.tile([128, 2], dtf)
            nc.sync.dma_start(iotac[:], iotac_d)
            W1s = const.tile([F, F], dth)
            nc.sync.dma_start(W1s[:], W1_d)
            W2s = const.tile([F, F], dth)
            nc.sync.dma_start(W2s[:], W2_d)
            Wls = const.tile([F, 1], dth)
            nc.sync.dma_start(Wls[:], Wl_d)
            ones1 = const.tile([1, 128], dth)
            nc.sync.dma_start(ones1[:], ones_d)
            b1row = const.tile([1, F], dth)
            nc.sync.dma_start(b1row[:], b1r_d)
            b2s = const.tile([F, 1], dtf)
            nc.sync.dma_start(b2s[:], b2_d)
            bls = const.tile([1, 1], dtf)
            nc.sync.dma_start(bls[:], bl_d)
            dinv2s = const.tile([128, T], dtf)
            nc.sync.dma_start(dinv2s[:], dinv2_d)
            dsts = const.tile([128, Ctot], dtf)
            nc.sync.dma_start(dsts[:], dst_d)
            norms = const.tile([128, Ctot], dtf)
            nc.sync.dma_start(norms[:], nrm_d)
            idxs = const.tile([128, Ctot * 8], mybir.dt.int16)
            nc.sync.dma_start(idxs[:], idx_d)

            outsb = const.tile([1, S], dtf)

            h1_loc = dram.tile([S, F], dth)
            ag_tab = dram.tile([cfg.NSLOT, F], dth, addr_space="Shared")

            qctr = 0
            for layer in range(2):
                table = xtab_d if layer == 0 else ag_tab[:]
                loc = xloc_d if layer == 0 else h1_loc[:]
                Ws = W1s if layer == 0 else W2s

                st0 = 0
                for bi, bsz in enumerate(cfg.BLOCKS):
                    blk = sb.tile([128, 2 * cfg.BSTM, F], dth, tag="selfblk")
                    nc.sync.dma_start(
                        blk[:, :2 * bsz, :],
                        loc[st0 * STW:st0 * STW + bsz * STW, :]
                        .rearrange("(bt p) f -> p bt f", p=128))

                    psums = {}
                    for sti in range(bsz):
                        st = st0 + sti
                        ps = aggp.tile([128, STW], dtf, tag=f"agg{sti}",
                                       name=f"agg{sti}")
                        psums[st] = ps
                        for i in range(2):
                            t = 2 * st + i
                            soh = ohp.tile([128, STW], dth, tag="soh")
                            nc.vector.tensor_scalar(
                                out=soh[:], in0=iota256[:],
                                scalar1=iotac[:, i:i + 1],
                                scalar2=dinv2s[:, t:t + 1],
                                op0=EQ, op1=MUL)
                            is_last = (st not in last_of_st) and i == 1
                            nc.tensor.matmul(
                                out=ps[:], lhsT=blk[:, 2 * sti + i, :],
                                rhs=soh[:], start=(i == 0), stop=is_last,
                                skip_group_check=True)

                    for w in range(NW):
                        for call in [cl for cl in calls
                                     if cl[0] == bi and cl[1] == w]:
                            _, _, col0, chunks = call
                            ncols = len(chunks)
                            q = qctr % 4
                            qctr += 1
                            xb = sb.tile([128, GB // 128, F], dth,
                                         tag=f"xb{q}", bufs=1)
                            nc.gpsimd.dma_gather(
                                xb[:, :ncols, :],
                                table[w * WIN:(w + 1) * WIN, :],
                                idxs[:, col0 * 8:(col0 + ncols) * 8],
                                ncols * 128, ncols * 128, F,
                                single_packet=False, queue_num=q)
                            for j, (st, k) in enumerate(chunks):
                                gch = int(chunk_pos[st, w]) + k
                                oh = ohp.tile([128, STW], dth, tag="oh")
                                nc.vector.tensor_scalar(
                                    out=oh[:], in0=iota256[:],
                                    scalar1=dsts[:, gch:gch + 1],
                                    scalar2=norms[:, gch:gch + 1],
                                    op0=EQ, op1=MUL)
                                is_last = last_of_st.get(st) == (w, k)
                                nc.tensor.matmul(
                                    out=psums[st][:], lhsT=xb[:, j, :],
                                    rhs=oh[:], start=False, stop=is_last,
                                    skip_group_check=True)

                    # block transforms
                    for sti in range(bsz):
                        st = st0 + sti
                        aggb = sb.tile([128, STW], dth, tag="aggb")
                        nc.vector.tensor_copy(out=aggb[:], in_=psums[st][:])
                        pp = pt.tile([128, STW], dtf, tag="pp", bufs=2)
                        if layer == 0:
                            # node-major: pp[:, i*128+f] = agg_i.T @ W1 + b1
                            for i in range(2):
                                sl = slice(i * 128, (i + 1) * 128)
                                nc.tensor.matmul(
                                    out=pp[:, sl],
                                    lhsT=aggb[:, sl], rhs=Ws[:],
                                    start=True, stop=False,
                                    skip_group_check=True)
                                nc.tensor.matmul(
                                    out=pp[:, sl], lhsT=ones1[:],
                                    rhs=b1row[:], start=False, stop=True,
                                    skip_group_check=True)
                            for i in range(2):
                                t = 2 * st + i
                                sl = slice(i * 128, (i + 1) * 128)
                                h1n = sb.tile([128, F], dth, tag="h1n")
                                nc.scalar.activation(out=h1n[:],
                                                     in_=pp[:, sl], func=Relu)
                                nc.sync.dma_start(
                                    h1_loc[t * 128:(t + 1) * 128, :], h1n[:])
                        else:
                            # feature-major: pp = W2.T @ agg (256 wide)
                            nc.tensor.matmul(out=pp[:], lhsT=Ws[:],
                                             rhs=aggb[:], start=True,
                                             stop=True, skip_group_check=True)
                            h2t = sb.tile([128, STW], dth, tag="h2t")
                            nc.scalar.activation(out=h2t[:], in_=pp[:],
                                                 func=Relu, bias=b2s[:])
                            p4 = pt.tile([1, STW], dtf, tag="p4", bufs=1)
                            nc.tensor.matmul(out=p4[:], lhsT=Wls[:],
                                             rhs=h2t[:], start=True,
                                             stop=True, skip_group_check=True)
                            nc.vector.tensor_scalar(
                                out=outsb[:, st * STW:(st + 1) * STW],
                                in0=p4[:], scalar1=bls[:], scalar2=None,
                                op0=ADD)
                    st0 += bsz

                if layer == 0:
                    nc.gpsimd.collective_compute(
                        "AllGather", mybir.AluOpType.bypass,
                        replica_groups=[list(range(C))],
                        ins=[h1_loc[:]], outs=[ag_tab[:]])

            nc.sync.dma_start(out_d, outsb[:])

    nc.compile()
    return nc


# ------------------------------------------------------------------ entry ----

def make_in_maps(cfg, per_core, W1, b1, W2, b2, Wl, bl):
    iota256 = np.tile(np.arange(STW, dtype=np.float16), (128, 1))
    iotac = np.stack([np.arange(128, dtype=np.float32),
                      np.arange(128, dtype=np.float32) + 128], axis=1)
    maps = []
    for c in range(cfg.C):
        pc = per_core[c]
        maps.append(dict(
            xtab=pc["xtab"], xloc=pc["xloc"], idx_w=pc["idx_w"],
            dst_t=pc["dst_t"], nrm_t=pc["nrm_t"], dinv2_t=pc["dinv2_t"],
            iota256=iota256, iota_col2=iotac,
            ones1=np.ones((1, 128), dtype=np.float16),
            b1row=np.asarray(b1, np.float16).reshape(1, F),
            W1=np.asarray(W1, np.float16), W2=np.asarray(W2, np.float16),
            Wl=np.asarray(Wl, np.float16).reshape(F, 1),
            b2=np.asarray(b2, np.float32).reshape(F, 1),
            bl=np.asarray(bl, np.float32).reshape(1, 1),
        ))
    return maps


def run(cfg, x, edge_index, W1, b1, W2, b2, Wl, bl, trace=False, nc=None):
    from concourse import bass_utils

    layout, per_core = prepare(cfg, x, edge_index)
    if nc is None:
        nc = build_nc(cfg, layout)
    in_maps = make_in_maps(cfg, per_core, W1, b1, W2, b2, Wl, bl)
    res = bass_utils.run_bass_kernel_spmd(nc, in_maps,
                                          core_ids=list(range(cfg.C)),
                                          trace=trace)
    out = np.concatenate([res.results[c]["out"][0, :cfg.NR]
                          for c in range(cfg.C)])
    return out.astype(np.float32), res


def kernel(x, edge_index, W1, b1, W2, b2, Wl, bl):
    out, _ = run(FULL, x, edge_index, W1, b1, W2, b2, Wl, bl)
    return out


# revision 8
# speedup vs baseline: 1.4380x; 1.2077x over previous
"""GCN (2-layer GCNConv + linear head) distributed over 8 TRN2 NeuronCores.

v3 strategy (dst-partitioned graph parallel, fp16 hot path, 4 SWDGE queues):
  - Nodes partitioned into 8 contiguous ranges; core c owns the scatter
    destinations for its range. Per-edge messages gathered from a node-major
    fp16 table in DRAM via dma_gather (256B rows) round-robined over 4 SWDGE
    queues (4x the single-queue ring drain rate).
  - GCN normalization is folded into the data so the scatter weights are
    pure 0/1: the gather table holds x' = x * dinv (host) resp.
    h1' = dinv * relu(...) (device), and the remaining per-dst dinv factor
    rides the transform activation's per-partition scale. Self-loops become
    plain edges (constant diagonal one-hot, no DVE work).
  - Scatter via one-hot matmul on the TensorEngine: for each chunk of 128
    gathered edges, rhs[e, d] = (d == dst_e) is ONE fused DVE tensor_scalar
    (is_equal against a per-partition scalar); psum[f, d] += chunk.T @ rhs.
  - Cells are (supertile=256 dsts, window=src range); chunks padded to 128
    per cell with counts maxed across cores (shared SPMD schedule). The
    idx/dst streams are shared by both layers and kept resident in SBUF.
  - Per-block pipeline: up to 5 supertile PSUM accumulators (1 bank each)
    live across the block's windows; transforms chase the accumulation.
    Both layers' transforms are NODE-major (lhsT=agg, rhs=W) so the
    per-dst dinv scale is a per-partition activation scale; layer-2's
    linear head is one DVE tensor_tensor_reduce per tile.
  - One AllGather (fp16, ~25MB) between the layers.

Host prep (degree/norm computation, edge bucketing, padding) is numpy inside
kernel(); the device kernel is a single static SPMD program.
"""

import math
import os
import sys

import numpy as np

for _p in ("/opt/trn_rl_repo",):
    if _p not in sys.path and os.path.isdir(_p):
        sys.path.insert(0, _p)

F = 128  # feature/hidden width
STW = 256  # dsts per supertile (2 tiles)


class Cfg:
    def __init__(self, n_cores=8, nodes_real_per_core=12500, n_edges=1_600_000,
                 n_windows=4, bst_max=5, gather_rows=4096):
        self.C = n_cores
        self.NR = nodes_real_per_core
        self.T = (self.NR + 127) // 128          # dst tiles per core
        assert self.T % 2 == 0, "supertile=2 tiles needs even T"
        self.NST = self.T // 2                   # supertiles per core
        self.S = self.T * 128                    # node slots per core
        self.NSLOT = self.C * self.S             # global slot count
        self.NW = n_windows
        assert self.NSLOT % self.NW == 0
        self.WIN = self.NSLOT // self.NW         # rows per gather window
        assert self.WIN <= 32767, "dma_gather idx is int16"
        self.BSTM = bst_max                      # max supertiles per block
        self.BLOCKS = []
        r = self.NST
        while r > 0:
            self.BLOCKS.append(min(bst_max, r))
            r -= min(bst_max, r)
        self.GB = gather_rows                    # max rows per dma_gather
        assert self.GB % 128 == 0
        self.N = self.C * self.NR                # real node count
        self.E = n_edges


FULL = Cfg()


# ------------------------------------------------------------- host prep ----

def prepare(cfg: Cfg, x, edge_index):
    """Compute per-core device inputs + the shared static schedule."""
    C, NR, T, S, NW, WIN = cfg.C, cfg.NR, cfg.T, cfg.S, cfg.NW, cfg.WIN
    NST, GB = cfg.NST, cfg.GB
    N = cfg.N
    src = np.asarray(edge_index[0], dtype=np.int64)
    dst = np.asarray(edge_index[1], dtype=np.int64)
    x = np.asarray(x, dtype=np.float32)

    deg = np.bincount(dst, minlength=N).astype(np.float64) + 1.0  # + self loop
    dinv = (1.0 / np.sqrt(deg)).astype(np.float32)

    core_of = dst // NR
    s_slot = S * (src // NR) + (src % NR)
    d_in_core = dst % NR
    st_of = d_in_core // STW
    d_loc = (d_in_core - st_of * STW).astype(np.float32)  # 0..255
    w_of = s_slot // WIN

    order = np.lexsort((s_slot, w_of, st_of, core_of))
    s_slot, d_loc = s_slot[order], d_loc[order]
    core_s, st_s, w_s = core_of[order], st_of[order], w_of[order]

    cell = ((core_s * NST + st_s) * NW + w_s).astype(np.int64)
    counts = np.bincount(cell, minlength=C * NST * NW).reshape(C, NST, NW)
    Kcell = np.ceil(counts / 128.0).astype(np.int64).max(axis=0)  # [NST, NW]
    cell_starts = np.zeros(C * NST * NW + 1, dtype=np.int64)
    np.cumsum(np.bincount(cell, minlength=C * NST * NW), out=cell_starts[1:])

    # ---- shared schedule: blocks x windows -> calls of <= GB/128 chunks ----
    chunk_pos = np.zeros((NST, NW), dtype=np.int64)
    calls = []  # (block_idx, w, col0_chunk, [(st, k)...]) per gather call
    acc = 0
    st0 = 0
    for bi, bsz in enumerate(cfg.BLOCKS):
        sts = range(st0, st0 + bsz)
        for w in range(NW):
            pend = []
            for st in sts:
                chunk_pos[st, w] = acc + len(pend)
                pend += [(st, k) for k in range(int(Kcell[st, w]))]
            i = 0
            while i < len(pend):
                n = min(GB // 128, len(pend) - i)
                calls.append((bi, w, acc + i, pend[i:i + n]))
                i += n
            acc += len(pend)
        st0 += bsz
    Ctot = acc
    assert Ctot == int(Kcell.sum())

    # last (w, k) per supertile for the psum stop flag
    last_of_st = {}
    for st in range(NST):
        ws = [w for w in range(NW) if Kcell[st, w] > 0]
        if ws:
            w = ws[-1]
            last_of_st[st] = (w, int(Kcell[st, w]) - 1)

    # ---- per-core streams ----
    per_core = []
    for c in range(C):
        idxs = np.zeros(Ctot * 128, dtype=np.int16)
        dstv = np.full(Ctot * 128, -1.0, dtype=np.float32)
        for st in range(NST):
            for w in range(NW):
                K = int(Kcell[st, w])
                if K == 0:
                    continue
                ci = (c * NST + st) * NW + w
                e0, e1 = cell_starts[ci], cell_starts[ci + 1]
                n = int(e1 - e0)
                off = int(chunk_pos[st, w]) * 128
                idxs[off:off + n] = (s_slot[e0:e1] - w_s[e0:e1] * WIN).astype(np.int16)
                dstv[off:off + n] = d_loc[e0:e1]

        # idx wrapped [128, Ctot*8]: idx i -> [i%16, i//16], replicated x8
        idx_w = np.tile(idxs.reshape(-1, 16).T, (8, 1)).copy()
        dst_t = dstv.reshape(Ctot, 128).T.copy()

        dv = np.zeros(S, dtype=np.float32)
        dv[:NR] = dinv[c * NR:(c + 1) * NR]
        dinvc_t = dv.reshape(T, 128).T.copy()       # [128, T] dinv
        dinv2_t = (dv * dv).reshape(T, 128).T.copy()  # [128, T] dinv^2

        per_core.append(dict(idx_w=idx_w, dst_t=dst_t,
                             dinvc_t=dinvc_t, dinv2_t=dinv2_t))

    # node tables in slot space, pre-scaled by dinv (fp16)
    x_slot = np.zeros((cfg.NSLOT, F), dtype=np.float16)
    sl = S * (np.arange(N) // NR) + (np.arange(N) % NR)
    x_slot[sl] = (x * dinv[:, None]).astype(np.float16)
    for c in range(C):
        per_core[c]["xtab"] = x_slot
        per_core[c]["xloc"] = x_slot[c * S:(c + 1) * S].copy()

    layout = dict(Kcell=Kcell, calls=calls, chunk_pos=chunk_pos,
                  last_of_st=last_of_st, Ctot=Ctot)
    return layout, per_core


# ---------------------------------------------------------------- builder ----

def build_nc(cfg: Cfg, layout, zero_b1=True, zero_b2=True):
    import concourse.bacc as bacc
    import concourse.mybir as mybir
    import concourse.tile as tile

    dtf = mybir.dt.float32
    dth = mybir.dt.float16
    Relu = mybir.ActivationFunctionType.Relu
    Copy = mybir.ActivationFunctionType.Copy
    EQ = mybir.AluOpType.is_equal
    MUL = mybir.AluOpType.mult
    ADD = mybir.AluOpType.add
    MAX = mybir.AluOpType.max

    C, T, S, NW, WIN, GB = cfg.C, cfg.T, cfg.S, cfg.NW, cfg.WIN, cfg.GB
    NST = cfg.NST
    Kcell, calls, chunk_pos, last_of_st, Ctot = (
        layout["Kcell"], layout["calls"], layout["chunk_pos"],
        layout["last_of_st"], layout["Ctot"])

    nc = bacc.Bacc("TRN2", target_bir_lowering=False, debug=False,
                   num_devices=C, num_swdge_queues=4)

    xtab_d = nc.dram_tensor("xtab", [cfg.NSLOT, F], dth, kind="ExternalInput").ap()
    xloc_d = nc.dram_tensor("xloc", [S, F], dth, kind="ExternalInput").ap()
    idx_d = nc.dram_tensor("idx_w", [128, Ctot * 8], mybir.dt.int16,
                           kind="ExternalInput").ap()
    dst_d = nc.dram_tensor("dst_t", [128, Ctot], dtf, kind="ExternalInput").ap()
    dinvc_d = nc.dram_tensor("dinvc_t", [128, T], dtf, kind="ExternalInput").ap()
    dinv2_d = nc.dram_tensor("dinv2_t", [128, T], dtf, kind="ExternalInput").ap()
    iota_d = nc.dram_tensor("iota256", [128, STW], dth, kind="ExternalInput").ap()
    diag_d = nc.dram_tensor("diag2", [128, 2 * STW], dth, kind="ExternalInput").ap()
    W1_d = nc.dram_tensor("W1", [F, F], dth, kind="ExternalInput").ap()
    W2_d = nc.dram_tensor("W2", [F, F], dth, kind="ExternalInput").ap()
    Wlbc_d = nc.dram_tensor("Wlbc", [128, F], dth, kind="ExternalInput").ap()
    b1bc_d = nc.dram_tensor("b1bc", [128, F], dth, kind="ExternalInput").ap()
    b2bc_d = nc.dram_tensor("b2bc", [128, F], dth, kind="ExternalInput").ap()
    blc_d = nc.dram_tensor("blcol", [128, 1], dtf, kind="ExternalInput").ap()
    out_d = nc.dram_tensor("out", [S, 1], dtf, kind="ExternalOutput").ap()

    with tile.TileContext(nc) as tc:
        with (
            tc.tile_pool(name="const", bufs=1) as const,
            tc.tile_pool(name="sb", bufs=2) as sb,
            tc.tile_pool(name="ohp", bufs=6) as ohp,
            tc.tile_pool(name="aggp", bufs=1, space="PSUM") as aggp,
            tc.tile_pool(name="pt", bufs=1, space="PSUM") as pt,
            tc.tile_pool(name="dram", bufs=1, space="DRAM") as dram,
        ):
            iota256 = const.tile([128, STW], dth)
            nc.sync.dma_start(iota256[:], iota_d)
            diag2 = const.tile([128, 2 * STW], dth)
            nc.sync.dma_start(diag2[:], diag_d)
            W1s = const.tile([F, F], dth)
            nc.sync.dma_start(W1s[:], W1_d)
            W2s = const.tile([F, F], dth)
            nc.sync.dma_start(W2s[:], W2_d)
            Wlbc = const.tile([128, F], dth)
            nc.sync.dma_start(Wlbc[:], Wlbc_d)
            b1bc = const.tile([128, F], dth)
            nc.sync.dma_start(b1bc[:], b1bc_d)
            b2bc = const.tile([128, F], dth)
            nc.sync.dma_start(b2bc[:], b2bc_d)
            blcol = const.tile([128, 1], dtf)
            nc.sync.dma_start(blcol[:], blc_d)
            dinvcs = const.tile([128, T], dtf)
            nc.sync.dma_start(dinvcs[:], dinvc_d)
            dinv2s = const.tile([128, T], dtf)
            nc.sync.dma_start(dinv2s[:], dinv2_d)
            dsts = const.tile([128, Ctot], dtf)
            nc.sync.dma_start(dsts[:], dst_d)
            idxs = const.tile([128, Ctot * 8], mybir.dt.int16)
            nc.sync.dma_start(idxs[:], idx_d)

            outcols = const.tile([128, T], dtf)

            h1_loc = dram.tile([S, F], dth)
            ag_tab = dram.tile([cfg.NSLOT, F], dth, addr_space="Shared")

            qctr = 0
            for layer in range(2):
                table = xtab_d if layer == 0 else ag_tab[:]
                loc = xloc_d if layer == 0 else h1_loc[:]
                Ws = W1s if layer == 0 else W2s
                zb = zero_b1 if layer == 0 else zero_b2
                bbc = b1bc if layer == 0 else b2bc

                st0 = 0
                for bi, bsz in enumerate(cfg.BLOCKS):
                    blk = sb.tile([128, 2 * cfg.BSTM, F], dth, tag="selfblk")
                    nc.sync.dma_start(
                        blk[:, :2 * bsz, :],
                        loc[st0 * STW:st0 * STW + bsz * STW, :]
                        .rearrange("(bt p) f -> p bt f", p=128))

                    psums = {}
                    for sti in range(bsz):
                        st = st0 + sti
                        ps = aggp.tile([128, STW], dtf, tag=f"agg{sti}",
                                       name=f"agg{sti}")
                        psums[st] = ps
                        for i in range(2):
                            is_last = (st not in last_of_st) and i == 1
                            nc.tensor.matmul(
                                out=ps[:], lhsT=blk[:, 2 * sti + i, :],
                                rhs=diag2[:, i * STW:(i + 1) * STW],
                                start=(i == 0), stop=is_last,
                                skip_group_check=True)

                    for w in range(NW):
                        for call in [cl for cl in calls
                                     if cl[0] == bi and cl[1] == w]:
                            _, _, col0, chunks = call
                            ncols = len(chunks)
                            q = qctr % 4
                            qctr += 1
                            xb = sb.tile([128, GB // 128, F], dth,
                                         tag=f"xb{q}", bufs=2)
                            nc.gpsimd.dma_gather(
                                xb[:, :ncols, :],
                                table[w * WIN:(w + 1) * WIN, :],
                                idxs[:, col0 * 8:(col0 + ncols) * 8],
                                ncols * 128, ncols * 128, F,
                                single_packet=False, queue_num=q)
                            for j, (st, k) in enumerate(chunks):
                                gch = int(chunk_pos[st, w]) + k
                                oh = ohp.tile([128, STW], dth, tag="oh")
                                nc.vector.tensor_scalar(
                                    out=oh[:], in0=iota256[:],
                                    scalar1=dsts[:, gch:gch + 1],
                                    scalar2=1.0, op0=EQ, op1=MUL)
                                is_last = last_of_st.get(st) == (w, k)
                                nc.tensor.matmul(
                                    out=psums[st][:], lhsT=xb[:, j, :],
                                    rhs=oh[:], start=False, stop=is_last,
                                    skip_group_check=True)

                    # block transforms (node-major both layers)
                    for sti in range(bsz):
                        st = st0 + sti
                        aggb = sb.tile([128, STW], dth, tag="aggb")
                        nc.vector.tensor_copy(out=aggb[:], in_=psums[st][:])
                        pp = pt.tile([128, STW], dtf, tag="pp", bufs=2)
                        for i in range(2):
                            sl = slice(i * 128, (i + 1) * 128)
                            nc.tensor.matmul(
                                out=pp[:, sl], lhsT=aggb[:, sl], rhs=Ws[:],
                                start=True, stop=True, skip_group_check=True)
                        for i in range(2):
                            t = 2 * st + i
                            sl = slice(i * 128, (i + 1) * 128)
                            if layer == 0:
                                h1n = sb.tile([128, F], dth, tag="h1n")
                                if zb:
                                    # h1' = dinv*relu(dinv*z) = relu(dinv^2 z)
                                    nc.scalar.activation(
                                        out=h1n[:], in_=pp[:, sl], func=Relu,
                                        scale=dinv2s[:, t:t + 1])
                                else:
                                    u = sb.tile([128, F], dth, tag="u")
                                    nc.scalar.activation(
                                        out=u[:], in_=pp[:, sl], func=Copy,
                                        scale=dinvcs[:, t:t + 1])
                                    nc.vector.tensor_tensor(
                                        out=u[:], in0=u[:], in1=b1bc[:],
                                        op=ADD)
                                    nc.scalar.activation(
                                        out=h1n[:], in_=u[:], func=Relu,
                                        scale=dinvcs[:, t:t + 1])
                                nc.sync.dma_start(
                                    h1_loc[t * 128:(t + 1) * 128, :], h1n[:])
                            else:
                                h2n = sb.tile([128, F], dth, tag="h2n")
                                if zb:
                                    nc.scalar.activation(
                                        out=h2n[:], in_=pp[:, sl], func=Relu,
                                        scale=dinvcs[:, t:t + 1])
                                else:
                                    u = sb.tile([128, F], dth, tag="u")
                                    nc.scalar.activation(
                                        out=u[:], in_=pp[:, sl], func=Copy,
                                        scale=dinvcs[:, t:t + 1])
                                    nc.vector.tensor_tensor(
                                        out=u[:], in0=u[:], in1=b2bc[:],
                                        op=ADD)
                                    nc.vector.tensor_scalar(
                                        out=h2n[:], in0=u[:], scalar1=0.0,
                                        scalar2=None, op0=MAX)
                                scr = sb.tile([128, F], dth, tag="scr")
                                nc.vector.tensor_tensor(
                                    out=scr[:], in0=h2n[:], in1=Wlbc[:],
                                    op=MUL)
                                nc.vector.tensor_reduce(
                                    out=outcols[:, t:t + 1], in_=scr[:],
                                    axis=mybir.AxisListType.X, op=ADD)
                    st0 += bsz

                if layer == 0:
                    nc.gpsimd.collective_compute(
                        "AllGather", mybir.AluOpType.bypass,
                        replica_groups=[list(range(C))],
                        ins=[h1_loc[:]], outs=[ag_tab[:]])

            nc.vector.tensor_scalar(out=outcols[:], in0=outcols[:],
                                    scalar1=blcol[:], scalar2=None, op0=ADD)
            nc.sync.dma_start(
                out_d.rearrange("(t p) o -> p (t o)", p=128), outcols[:])

    nc.compile()
    return nc


# ------------------------------------------------------------------ entry ----

def make_in_maps(cfg, per_core, W1, b1, W2, b2, Wl, bl):
    iota256 = np.tile(np.arange(STW, dtype=np.float16), (128, 1))
    diag2 = np.zeros((128, 2 * STW), dtype=np.float16)
    for i in range(2):
        for p in range(128):
            diag2[p, i * STW + p + 128 * i] = 1.0
    maps = []
    for c in range(cfg.C):
        pc = per_core[c]
        maps.append(dict(
            xtab=pc["xtab"], xloc=pc["xloc"], idx_w=pc["idx_w"],
            dst_t=pc["dst_t"], dinvc_t=pc["dinvc_t"], dinv2_t=pc["dinv2_t"],
            iota256=iota256, diag2=diag2,
            W1=np.asarray(W1, np.float16), W2=np.asarray(W2, np.float16),
            Wlbc=np.tile(np.asarray(Wl, np.float16).reshape(1, F), (128, 1)),
            b1bc=np.tile(np.asarray(b1, np.float16).reshape(1, F), (128, 1)),
            b2bc=np.tile(np.asarray(b2, np.float16).reshape(1, F), (128, 1)),
            blcol=np.full((128, 1), np.asarray(bl, np.float32).reshape(-1)[0],
                          dtype=np.float32),
        ))
    return maps


def run(cfg, x, edge_index, W1, b1, W2, b2, Wl, bl, trace=False, nc=None):
    from concourse import bass_utils

    layout, per_core = prepare(cfg, x, edge_index)
    if nc is None:
        nc = build_nc(cfg, layout,
                      zero_b1=not np.any(np.asarray(b1)),
                      zero_b2=not np.any(np.asarray(b2)))
    in_maps = make_in_maps(cfg, per_core, W1, b1, W2, b2, Wl, bl)
    res = bass_utils.run_bass_kernel_spmd(nc, in_maps,
                                          core_ids=list(range(cfg.C)),
                                          trace=trace)
    out = np.concatenate([res.results[c]["out"][:cfg.NR, 0]
                          for c in range(cfg.C)])
    return out.astype(np.float32), res


def kernel(x, edge_index, W1, b1, W2, b2, Wl, bl):
    out, _ = run(FULL, x, edge_index, W1, b1, W2, b2, Wl, bl)
    return out


# revision 14
# speedup vs baseline: 1.4471x; 1.0063x over previous
"""GCN (2-layer GCNConv + linear head) distributed over 8 TRN2 NeuronCores.

v3 strategy (dst-partitioned graph parallel, fp16 hot path, 4 SWDGE queues):
  - Nodes partitioned into 8 contiguous ranges; core c owns the scatter
    destinations for its range. Per-edge messages gathered from a node-major
    fp16 table in DRAM via dma_gather (256B rows) round-robined over 4 SWDGE
    queues (4x the single-queue ring drain rate).
  - GCN normalization is folded into the data so the scatter weights are
    pure 0/1: the gather table holds x' = x * dinv (host) resp.
    h1' = dinv * relu(...) (device), and the remaining per-dst dinv factor
    rides the transform activation's per-partition scale. Self-loops become
    plain edges (constant diagonal one-hot, no DVE work).
  - Scatter via one-hot matmul on the TensorEngine: for each chunk of 128
    gathered edges, rhs[e, d] = (d == dst_e) is ONE fused DVE tensor_scalar
    (is_equal against a per-partition scalar); psum[f, d] += chunk.T @ rhs.
  - Cells are (supertile=256 dsts, window=src range); chunks padded to 128
    per cell with counts maxed across cores (shared SPMD schedule). The
    idx/dst streams are shared by both layers and kept resident in SBUF.
  - Per-block pipeline: up to 5 supertile PSUM accumulators (1 bank each)
    live across the block's windows; transforms chase the accumulation.
    Both layers' transforms are NODE-major (lhsT=agg, rhs=W) so the
    per-dst dinv scale is a per-partition activation scale; layer-2's
    linear head is one DVE tensor_tensor_reduce per tile.
  - One AllGather (fp16, ~25MB) between the layers.

Host prep (degree/norm computation, edge bucketing, padding) is numpy inside
kernel(); the device kernel is a single static SPMD program.
"""

import math
import os
import sys

import numpy as np

for _p in ("/opt/trn_rl_repo",):
    if _p not in sys.path and os.path.isdir(_p):
        sys.path.insert(0, _p)

F = 128  # feature/hidden width
STW = 256  # dsts per supertile (2 tiles)


class Cfg:
    def __init__(self, n_cores=8, nodes_real_per_core=12500, n_edges=1_600_000,
                 n_windows=4, bst_max=5, gather_rows=4096):
        self.C = n_cores
        self.NR = nodes_real_per_core
        self.T = (self.NR + 127) // 128          # dst tiles per core
        assert self.T % 2 == 0, "supertile=2 tiles needs even T"
        self.NST = self.T // 2                   # supertiles per core
        self.S = self.T * 128                    # node slots per core
        self.NSLOT = self.C * self.S             # global slot count
        self.NW = n_windows
        assert self.NSLOT % self.NW == 0
        self.WIN = self.NSLOT // self.NW         # rows per gather window
        assert self.WIN <= 32767, "dma_gather idx is int16"
        self.BSTM = bst_max                      # max supertiles per block
        self.BLOCKS = []
        r = self.NST
        while r > 0:
            self.BLOCKS.append(min(bst_max, r))
            r -= min(bst_max, r)
        self.GB = gather_rows                    # max rows per dma_gather
        assert self.GB % 128 == 0
        self.N = self.C * self.NR                # real node count
        self.E = n_edges


FULL = Cfg()


# ------------------------------------------------------------- host prep ----

def prepare(cfg: Cfg, x, edge_index):
    """Compute per-core device inputs + the shared static schedule."""
    C, NR, T, S, NW, WIN = cfg.C, cfg.NR, cfg.T, cfg.S, cfg.NW, cfg.WIN
    NST, GB = cfg.NST, cfg.GB
    N = cfg.N
    src = np.asarray(edge_index[0], dtype=np.int64)
    dst = np.asarray(edge_index[1], dtype=np.int64)
    x = np.asarray(x, dtype=np.float32)

    deg = np.bincount(dst, minlength=N).astype(np.float64) + 1.0  # + self loop
    dinv = (1.0 / np.sqrt(deg)).astype(np.float32)

    core_of = dst // NR
    s_slot = S * (src // NR) + (src % NR)
    d_in_core = dst % NR
    st_of = d_in_core // STW
    d_loc = (d_in_core - st_of * STW).astype(np.float32)  # 0..255
    w_of = s_slot // WIN

    order = np.lexsort((s_slot, w_of, st_of, core_of))
    s_slot, d_loc = s_slot[order], d_loc[order]
    core_s, st_s, w_s = core_of[order], st_of[order], w_of[order]

    cell = ((core_s * NST + st_s) * NW + w_s).astype(np.int64)
    counts = np.bincount(cell, minlength=C * NST * NW).reshape(C, NST, NW)
    Kcell = np.ceil(counts / 128.0).astype(np.int64).max(axis=0)  # [NST, NW]
    cell_starts = np.zeros(C * NST * NW + 1, dtype=np.int64)
    np.cumsum(np.bincount(cell, minlength=C * NST * NW), out=cell_starts[1:])

    # ---- shared schedule: blocks x windows -> calls of <= GB/128 chunks ----
    chunk_pos = np.zeros((NST, NW), dtype=np.int64)
    calls = []  # (block_idx, w, col0_chunk, [(st, k)...]) per gather call
    acc = 0
    st0 = 0
    for bi, bsz in enumerate(cfg.BLOCKS):
        sts = range(st0, st0 + bsz)
        for w in range(NW):
            pend = []
            for st in sts:
                chunk_pos[st, w] = acc + len(pend)
                pend += [(st, k) for k in range(int(Kcell[st, w]))]
            i = 0
            while i < len(pend):
                n = min(GB // 128, len(pend) - i)
                calls.append((bi, w, acc + i, pend[i:i + n]))
                i += n
            acc += len(pend)
        st0 += bsz
    Ctot = acc
    assert Ctot == int(Kcell.sum())

    # last (w, k) per supertile for the psum stop flag
    last_of_st = {}
    for st in range(NST):
        ws = [w for w in range(NW) if Kcell[st, w] > 0]
        if ws:
            w = ws[-1]
            last_of_st[st] = (w, int(Kcell[st, w]) - 1)

    # ---- per-core streams ----
    per_core = []
    for c in range(C):
        idxs = np.zeros(Ctot * 128, dtype=np.int16)
        dstv = np.full(Ctot * 128, -1.0, dtype=np.float32)
        for st in range(NST):
            for w in range(NW):
                K = int(Kcell[st, w])
                if K == 0:
                    continue
                ci = (c * NST + st) * NW + w
                e0, e1 = cell_starts[ci], cell_starts[ci + 1]
                n = int(e1 - e0)
                off = int(chunk_pos[st, w]) * 128
                idxs[off:off + n] = (s_slot[e0:e1] - w_s[e0:e1] * WIN).astype(np.int16)
                dstv[off:off + n] = d_loc[e0:e1]

        # idx wrapped [128, Ctot*8]: idx i -> [i%16, i//16], replicated x8
        idx_w = np.tile(idxs.reshape(-1, 16).T, (8, 1)).copy()
        dst_t = dstv.reshape(Ctot, 128).T.copy()

        dv = np.zeros(S, dtype=np.float32)
        dv[:NR] = dinv[c * NR:(c + 1) * NR]
        dinvc_t = dv.reshape(T, 128).T.copy()       # [128, T] dinv
        dinv2_t = (dv * dv).reshape(T, 128).T.copy()  # [128, T] dinv^2

        per_core.append(dict(idx_w=idx_w, dst_t=dst_t,
                             dinvc_t=dinvc_t, dinv2_t=dinv2_t))

    # node tables in slot space, pre-scaled by dinv (fp16)
    x_slot = np.zeros((cfg.NSLOT, F), dtype=np.float16)
    sl = S * (np.arange(N) // NR) + (np.arange(N) % NR)
    x_slot[sl] = (x * dinv[:, None]).astype(np.float16)
    for c in range(C):
        per_core[c]["xtab"] = x_slot
        per_core[c]["xloc"] = x_slot[c * S:(c + 1) * S].copy()

    layout = dict(Kcell=Kcell, calls=calls, chunk_pos=chunk_pos,
                  last_of_st=last_of_st, Ctot=Ctot)
    return layout, per_core


# ---------------------------------------------------------------- builder ----

def build_nc(cfg: Cfg, layout, zero_b1=True, zero_b2=True):
    import concourse.bacc as bacc
    import concourse.mybir as mybir
    import concourse.tile as tile

    dtf = mybir.dt.float32
    dth = mybir.dt.float16
    Relu = mybir.ActivationFunctionType.Relu
    Copy = mybir.ActivationFunctionType.Copy
    EQ = mybir.AluOpType.is_equal
    MUL = mybir.AluOpType.mult
    ADD = mybir.AluOpType.add
    MAX = mybir.AluOpType.max

    C, T, S, NW, WIN, GB = cfg.C, cfg.T, cfg.S, cfg.NW, cfg.WIN, cfg.GB
    NST = cfg.NST
    Kcell, calls, chunk_pos, last_of_st, Ctot = (
        layout["Kcell"], layout["calls"], layout["chunk_pos"],
        layout["last_of_st"], layout["Ctot"])

    nc = bacc.Bacc("TRN2", target_bir_lowering=False, debug=False,
                   num_devices=C, num_swdge_queues=4)

    xtab_d = nc.dram_tensor("xtab", [cfg.NSLOT, F], dth, kind="ExternalInput").ap()
    xloc_d = nc.dram_tensor("xloc", [S, F], dth, kind="ExternalInput").ap()
    idx_d = nc.dram_tensor("idx_w", [128, Ctot * 8], mybir.dt.int16,
                           kind="ExternalInput").ap()
    dst_d = nc.dram_tensor("dst_t", [128, Ctot], dtf, kind="ExternalInput").ap()
    dinvc_d = nc.dram_tensor("dinvc_t", [128, T], dtf, kind="ExternalInput").ap()
    dinv2_d = nc.dram_tensor("dinv2_t", [128, T], dtf, kind="ExternalInput").ap()
    iota_d = nc.dram_tensor("iota256", [128, STW], dth, kind="ExternalInput").ap()
    diag_d = nc.dram_tensor("diag2", [128, 2 * STW], dth, kind="ExternalInput").ap()
    W1_d = nc.dram_tensor("W1", [F, F], dth, kind="ExternalInput").ap()
    W2_d = nc.dram_tensor("W2", [F, F], dth, kind="ExternalInput").ap()
    Wlbc_d = nc.dram_tensor("Wlbc", [128, F], dth, kind="ExternalInput").ap()
    b1bc_d = nc.dram_tensor("b1bc", [128, F], dth, kind="ExternalInput").ap()
    b2bc_d = nc.dram_tensor("b2bc", [128, F], dth, kind="ExternalInput").ap()
    blc_d = nc.dram_tensor("blcol", [128, 1], dtf, kind="ExternalInput").ap()
    out_d = nc.dram_tensor("out", [S, 1], dtf, kind="ExternalOutput").ap()

    with tile.TileContext(nc) as tc:
        with (
            tc.tile_pool(name="const", bufs=1) as const,
            tc.tile_pool(name="sb", bufs=2) as sb,
            tc.tile_pool(name="ohp", bufs=6) as ohp,
            tc.tile_pool(name="aggp", bufs=1, space="PSUM") as aggp,
            tc.tile_pool(name="pt", bufs=1, space="PSUM") as pt,
            tc.tile_pool(name="dram", bufs=1, space="DRAM") as dram,
        ):
            iota256 = const.tile([128, STW], dth)
            nc.sync.dma_start(iota256[:], iota_d)
            diag2 = const.tile([128, 2 * STW], dth)
            nc.sync.dma_start(diag2[:], diag_d)
            W1s = const.tile([F, F], dth)
            nc.sync.dma_start(W1s[:], W1_d)
            W2s = const.tile([F, F], dth)
            nc.sync.dma_start(W2s[:], W2_d)
            Wlbc = const.tile([128, F], dth)
            nc.sync.dma_start(Wlbc[:], Wlbc_d)
            b1bc = const.tile([128, F], dth)
            nc.sync.dma_start(b1bc[:], b1bc_d)
            b2bc = const.tile([128, F], dth)
            nc.sync.dma_start(b2bc[:], b2bc_d)
            blcol = const.tile([128, 1], dtf)
            nc.sync.dma_start(blcol[:], blc_d)
            dinvcs = const.tile([128, T], dtf)
            nc.sync.dma_start(dinvcs[:], dinvc_d)
            dinv2s = const.tile([128, T], dtf)
            nc.sync.dma_start(dinv2s[:], dinv2_d)
            dsts = const.tile([128, Ctot], dtf)
            nc.sync.dma_start(dsts[:], dst_d)
            idxs = const.tile([128, Ctot * 8], mybir.dt.int16)
            nc.sync.dma_start(idxs[:], idx_d)

            outcols = const.tile([128, T], dtf)

            h1_loc = dram.tile([S, F], dth)
            ag_tab = dram.tile([cfg.NSLOT, F], dth, addr_space="Shared")

            qctr = 0
            for layer in range(2):
                table = xtab_d if layer == 0 else ag_tab[:]
                loc = xloc_d if layer == 0 else h1_loc[:]
                Ws = W1s if layer == 0 else W2s
                zb = zero_b1 if layer == 0 else zero_b2
                bbc = b1bc if layer == 0 else b2bc

                st0 = 0
                for bi, bsz in enumerate(cfg.BLOCKS):
                    blk = sb.tile([128, 2 * cfg.BSTM, F], dth, tag="selfblk")
                    nc.sync.dma_start(
                        blk[:, :2 * bsz, :],
                        loc[st0 * STW:st0 * STW + bsz * STW, :]
                        .rearrange("(bt p) f -> p bt f", p=128))

                    psums = {}
                    for sti in range(bsz):
                        st = st0 + sti
                        ps = aggp.tile([128, STW], dtf, tag=f"agg{sti}",
                                       name=f"agg{sti}")
                        psums[st] = ps
                        for i in range(2):
                            is_last = (st not in last_of_st) and i == 1
                            nc.tensor.matmul(
                                out=ps[:], lhsT=blk[:, 2 * sti + i, :],
                                rhs=diag2[:, i * STW:(i + 1) * STW],
                                start=(i == 0), stop=is_last,
                                skip_group_check=True)

                    for w in range(NW):
                        for call in [cl for cl in calls
                                     if cl[0] == bi and cl[1] == w]:
                            _, _, col0, chunks = call
                            ncols = len(chunks)
                            q = qctr % 4
                            qctr += 1
                            xb = sb.tile([128, GB // 128, F], dth,
                                         tag=f"xb{q}", bufs=3)
                            nc.gpsimd.dma_gather(
                                xb[:, :ncols, :],
                                table[w * WIN:(w + 1) * WIN, :],
                                idxs[:, col0 * 8:(col0 + ncols) * 8],
                                ncols * 128, ncols * 128, F,
                                single_packet=False, queue_num=q)
                            for j, (st, k) in enumerate(chunks):
                                gch = int(chunk_pos[st, w]) + k
                                oh = ohp.tile([128, STW], dth, tag="oh")
                                nc.vector.tensor_scalar(
                                    out=oh[:], in0=iota256[:],
                                    scalar1=dsts[:, gch:gch + 1],
                                    scalar2=1.0, op0=EQ, op1=MUL)
                                is_last = last_of_st.get(st) == (w, k)
                                nc.tensor.matmul(
                                    out=psums[st][:], lhsT=xb[:, j, :],
                                    rhs=oh[:], start=False, stop=is_last,
                                    skip_group_check=True)

                    # block transforms (node-major both layers)
                    for sti in range(bsz):
                        st = st0 + sti
                        aggb = sb.tile([128, STW], dth, tag="aggb")
                        nc.vector.tensor_copy(out=aggb[:], in_=psums[st][:])
                        pp = pt.tile([128, STW], dtf, tag="pp", bufs=2)
                        for i in range(2):
                            sl = slice(i * 128, (i + 1) * 128)
                            nc.tensor.matmul(
                                out=pp[:, sl], lhsT=aggb[:, sl], rhs=Ws[:],
                                start=True, stop=True, skip_group_check=True)
                        for i in range(2):
                            t = 2 * st + i
                            sl = slice(i * 128, (i + 1) * 128)
                            if layer == 0:
                                h1n = sb.tile([128, F], dth, tag="h1n")
                                if zb:
                                    # h1' = dinv*relu(dinv*z) = relu(dinv^2 z)
                                    nc.scalar.activation(
                                        out=h1n[:], in_=pp[:, sl], func=Relu,
                                        scale=dinv2s[:, t:t + 1])
                                else:
                                    u = sb.tile([128, F], dth, tag="u")
                                    nc.scalar.activation(
                                        out=u[:], in_=pp[:, sl], func=Copy,
                                        scale=dinvcs[:, t:t + 1])
                                    nc.vector.tensor_tensor(
                                        out=u[:], in0=u[:], in1=b1bc[:],
                                        op=ADD)
                                    nc.scalar.activation(
                                        out=h1n[:], in_=u[:], func=Relu,
                                        scale=dinvcs[:, t:t + 1])
                                nc.sync.dma_start(
                                    h1_loc[t * 128:(t + 1) * 128, :], h1n[:])
                            else:
                                h2n = sb.tile([128, F], dth, tag="h2n")
                                if zb:
                                    nc.scalar.activation(
                                        out=h2n[:], in_=pp[:, sl], func=Relu,
                                        scale=dinvcs[:, t:t + 1])
                                else:
                                    u = sb.tile([128, F], dth, tag="u")
                                    nc.scalar.activation(
                                        out=u[:], in_=pp[:, sl], func=Copy,
                                        scale=dinvcs[:, t:t + 1])
                                    nc.vector.tensor_tensor(
                                        out=u[:], in0=u[:], in1=b2bc[:],
                                        op=ADD)
                                    nc.vector.tensor_scalar(
                                        out=h2n[:], in0=u[:], scalar1=0.0,
                                        scalar2=None, op0=MAX)
                                scr = sb.tile([128, F], dth, tag="scr")
                                nc.vector.tensor_tensor(
                                    out=scr[:], in0=h2n[:], in1=Wlbc[:],
                                    op=MUL)
                                nc.vector.tensor_reduce(
                                    out=outcols[:, t:t + 1], in_=scr[:],
                                    axis=mybir.AxisListType.X, op=ADD)
                    st0 += bsz

                if layer == 0:
                    nc.gpsimd.collective_compute(
                        "AllGather", mybir.AluOpType.bypass,
                        replica_groups=[list(range(C))],
                        ins=[h1_loc[:]], outs=[ag_tab[:]])

            nc.vector.tensor_scalar(out=outcols[:], in0=outcols[:],
                                    scalar1=blcol[:], scalar2=None, op0=ADD)
            nc.sync.dma_start(
                out_d.rearrange("(t p) o -> p (t o)", p=128), outcols[:])

    nc.compile()
    return nc


# ------------------------------------------------------------------ entry ----

def make_in_maps(cfg, per_core, W1, b1, W2, b2, Wl, bl):
    iota256 = np.tile(np.arange(STW, dtype=np.float16), (128, 1))
    diag2 = np.zeros((128, 2 * STW), dtype=np.float16)
    for i in range(2):
        for p in range(128):
            diag2[p, i * STW + p + 128 * i] = 1.0
    maps = []
    for c in range(cfg.C):
        pc = per_core[c]
        maps.append(dict(
            xtab=pc["xtab"], xloc=pc["xloc"], idx_w=pc["idx_w"],
            dst_t=pc["dst_t"], dinvc_t=pc["dinvc_t"], dinv2_t=pc["dinv2_t"],
            iota256=iota256, diag2=diag2,
            W1=np.asarray(W1, np.float16), W2=np.asarray(W2, np.float16),
            Wlbc=np.tile(np.asarray(Wl, np.float16).reshape(1, F), (128, 1)),
            b1bc=np.tile(np.asarray(b1, np.float16).reshape(1, F), (128, 1)),
            b2bc=np.tile(np.asarray(b2, np.float16).reshape(1, F), (128, 1)),
            blcol=np.full((128, 1), np.asarray(bl, np.float32).reshape(-1)[0],
                          dtype=np.float32),
        ))
    return maps


def run(cfg, x, edge_index, W1, b1, W2, b2, Wl, bl, trace=False, nc=None):
    from concourse import bass_utils

    layout, per_core = prepare(cfg, x, edge_index)
    if nc is None:
        nc = build_nc(cfg, layout,
                      zero_b1=not np.any(np.asarray(b1)),
                      zero_b2=not np.any(np.asarray(b2)))
    in_maps = make_in_maps(cfg, per_core, W1, b1, W2, b2, Wl, bl)
    res = bass_utils.run_bass_kernel_spmd(nc, in_maps,
                                          core_ids=list(range(cfg.C)),
                                          trace=trace)
    out = np.concatenate([res.results[c]["out"][:cfg.NR, 0]
                          for c in range(cfg.C)])
    return out.astype(np.float32), res


def kernel(x, edge_index, W1, b1, W2, b2, Wl, bl):
    out, _ = run(FULL, x, edge_index, W1, b1, W2, b2, Wl, bl)
    return out
